# revision 6
# baseline (speedup 1.0000x reference)
"""Trainium2 Bass kernel for nn_ContrastiveLossV2 (8-core SPMD).

Reference computation:
    z = l2norm(concat([emb_i, emb_j]))          # [8192, 128]
    sim = z @ z.T                               # [8192, 8192]
    loss = mean((sim - class_pairs)**2)

Instead of materializing sim and running an elementwise (sim-cp)^2 pass,
expand the square:

    sum((sim-cp)^2) = sum(sim^2) - 2*sum(sim*cp) + sum(cp^2)

  * sum(sim^2)  = ||Z^T Z||_F^2   (Gram trick; per-core G_c = Z_c^T Z_c over
                  its 1024 local rows, G = sum_c G_c summed on host in f64)
  * sum(sim*cp) = sum_{d,c} V[d,c] * zT[d,c]  with  V = Z_loc^T @ CP_loc —
                  computed by the TensorEngine directly from *row-major* cp
                  tiles (contraction over the local row dim). The dot against
                  zT is a fused scalar_tensor_tensor per PSUM chunk.
  * sum(cp^2)   = Square+accumulate passes, split between the scalar and
                  vector engines.

Row sharding: core c owns reps rows [c*1024, (c+1)*1024) and the matching
1024-row slice of class_pairs. Everything per-core-specific arrives as data,
so one SPMD program serves all 8 cores.

The kernel is bound by the per-NeuronCore HBM read limit (~360-390 GB/s):
32MB of class_pairs per core. Structure:
  * the 32MB class_pairs stream (SWDGE, f32->bf16 cast in flight) is the
    FIRST thing on the gpsimd queue; all small staging inputs (pre-staged
    bf16 embs, identity) load via the scalar-engine HWDGE queue in parallel.
  * column groups [2048,2048,2048,1024,1024]: full-width for bulk stream
    efficiency, narrower at the end so the post-stream compute tail is short.
  * each engine accumulates cp^2 partials into its OWN tile and the narrow
    groups alternate scalar/vector per tile. (A shared accumulator tile
    cross-engine-orders the accumulate ops and was measured to serialize the
    entire stream at ~297 GB/s.)
Host combines all partial tensors in f64.
"""

import numpy as np

import concourse.bacc as bacc
import concourse.mybir as mybir
import concourse.tile as tile
from concourse.bass_utils import run_bass_kernel_spmd

f32 = mybir.dt.float32
bf16 = mybir.dt.bfloat16
AF = mybir.ActivationFunctionType
OP = mybir.AluOpType

N_CORES = 8
N, D = 4096, 128
TWO_N = 2 * N                     # 8192
R_LOC = TWO_N // N_CORES          # 1024 rows per core
M_BLK = R_LOC // 128              # 8 local 128-row blocks
NCH = 512                         # matmul free-dim chunk (one PSUM bank)
GROUP_W = [2048, 2048, 2048, 1024, 512, 512]   # column group widths
N_GRP = len(GROUP_W)
N_CHUNK = TWO_N // NCH            # 16 psum chunks total
EPS = 1e-12
# how many of the 8 per-group cp^2 squares run on the vector engine in the
# full-width groups (the rest run on the scalar engine)
CP2_ON_DVE = 2

# per-(group, m) engine assignment for the cp^2 squares: full-width groups
# put CP2_ON_DVE of 8 on the vector engine, narrow groups alternate per tile
# so neither engine serializes the stream tail.
SQ_ON_DVE = []
for _w in GROUP_W:
    for _m in range(M_BLK):
        SQ_ON_DVE.append((_m % 2 == 0) if _w <= 1024 else (_m < CP2_ON_DVE))
N_SQ_V = sum(SQ_ON_DVE)
N_SQ_S = len(SQ_ON_DVE) - N_SQ_V

_cached = {}


def _build_module():
    nc = bacc.Bacc("TRN2", target_bir_lowering=False, debug=False,
                   num_devices=N_CORES)

    # The (small, replicated) embedding inputs are uploaded pre-staged by the
    # host: bf16, already arranged as [partition, block, d] with block n
    # holding row n*128+p on partition p — so the device DMA is a fully
    # contiguous per-partition read instead of a 512B/row gather.
    emb_i = nc.dram_tensor("emb_i", [128, N // 128, D], bf16,
                           kind="ExternalInput")
    emb_j = nc.dram_tensor("emb_j", [128, N // 128, D], bf16,
                           kind="ExternalInput")
    emb_loc = nc.dram_tensor("emb_loc", [128, M_BLK, D], bf16,
                             kind="ExternalInput")
    cp_loc = nc.dram_tensor("cp_loc", [R_LOC, TWO_N], f32, kind="ExternalInput")
    ident = nc.dram_tensor("ident", [128, 128], bf16, kind="ExternalInput")

    out_g = nc.dram_tensor("out_g", [128, 128], f32, kind="ExternalOutput")
    out_s = nc.dram_tensor("out_s", [128, N_SQ_S], f32, kind="ExternalOutput")
    out_v = nc.dram_tensor("out_v", [128, N_SQ_V], f32, kind="ExternalOutput")
    out_x = nc.dram_tensor("out_x", [128, N_CHUNK], f32, kind="ExternalOutput")

    with tile.TileContext(nc) as tc:
        with (
            tc.tile_pool(name="const", bufs=1) as const_pool,
            tc.tile_pool(name="persist", bufs=1) as persist,
            tc.tile_pool(name="stag", bufs=2) as stag_pool,
            tc.tile_pool(name="sq", bufs=1) as sq_pool,
            tc.tile_pool(name="norm", bufs=4) as norm_pool,
            tc.tile_pool(name="zc", bufs=2) as zc_pool,
            tc.tile_pool(name="cpt", bufs=22) as cp_pool,
            tc.tile_pool(name="tmp", bufs=4) as tmp_pool,
            tc.tile_pool(name="sqj", bufs=2) as sqj_pool,
            tc.tile_pool(name="psv", bufs=5, space="PSUM") as psv_pool,
            tc.tile_pool(name="pst", bufs=2, space="PSUM") as pst_pool,
            tc.tile_pool(name="psg", bufs=1, space="PSUM") as psg_pool,
        ):
            # zT[d, r] = normalized reps row r, transposed. bf16.
            zT = persist.tile([128, TWO_N], bf16)
            # local row-major z tiles (natural 128-row blocks), bf16.
            z_loc = persist.tile([128, M_BLK, 128], bf16)
            # per-engine partial-sum accumulators (separate tiles: a shared
            # tile would cross-engine-order the accumulate ops)
            acc_s = persist.tile([128, N_SQ_S], f32)
            acc_v = persist.tile([128, N_SQ_V], f32)
            acc_x = persist.tile([128, N_CHUNK], f32)

            # ---- class_pairs stream: FIRST ops on the SWDGE queue ----
            # (cast f32->bf16 in flight; descriptor generation for tile 0
            # begins the moment the gpsimd engine is up)
            cp_tiles = [[None] * M_BLK for _ in GROUP_W]
            col_bases = []
            cb = 0
            for w in GROUP_W:
                col_bases.append(cb)
                cb += w
            for gi, w in enumerate(GROUP_W):
                cbase = col_bases[gi]
                for m in range(M_BLK):
                    cpt = cp_pool.tile([128, w], bf16, tag="cpt",
                                       name="cpt", padded_shape=[128, 2048])
                    nc.gpsimd.dma_start(
                        out=cpt[:],
                        in_=cp_loc[m * 128:(m + 1) * 128, cbase:cbase + w])
                    cp_tiles[gi][m] = cpt

            # ---- staging inputs via the scalar-engine HWDGE queue ----
            stag_l = stag_pool.tile([128, M_BLK, 128], bf16, tag="stag_loc",
                                    name="stag_loc")
            nc.scalar.dma_start(out=stag_l[:], in_=emb_loc[:])
            stags = []
            for emb in (emb_i, emb_j):
                stag = stag_pool.tile([128, 32, 128], bf16, tag="stag",
                                      name="stag")
                nc.scalar.dma_start(out=stag[:], in_=emb[:])
                stags.append(stag)
            ident_sb = const_pool.tile([128, 128], bf16)
            nc.scalar.dma_start(out=ident_sb[:], in_=ident[:])

            def normalize(stag, n_chunks, z_out):
                """stag: [128, n_chunks, 128] bf16 staging; chunk n is the
                natural 128-row block n (row n*128+p on partition p). Writes
                the row-normalized bf16 result into z_out [128, n_chunks, 128]
                with a single broadcast multiply (f32 internal math)."""
                sq = sq_pool.tile([128, n_chunks, 128], f32, tag="sq",
                                  name="sq")
                nc.vector.tensor_tensor(sq[:], stag[:], stag[:], op=OP.mult)
                nsq = norm_pool.tile([128, n_chunks], f32, tag="nsq",
                                     name="nsq")
                nc.vector.tensor_reduce(nsq[:], sq[:], axis=mybir.AxisListType.X,
                                        op=OP.add)
                nrm = norm_pool.tile([128, n_chunks], f32, tag="nrm",
                                     name="nrm")
                nc.scalar.activation(nrm[:], nsq[:], AF.Sqrt)
                nc.vector.tensor_scalar_max(nrm[:], nrm[:], EPS)
                rec = norm_pool.tile([128, n_chunks], f32, tag="rec",
                                     name="rec")
                nc.vector.reciprocal(rec[:], nrm[:])
                rec_b = rec[:].rearrange("q n -> q n ()") \
                    .broadcast_to([128, n_chunks, 128])
                nc.vector.tensor_tensor(z_out, stag[:], rec_b, op=OP.mult)

            # ---- phase A: local row blocks (natural order) ----
            normalize(stag_l, M_BLK, z_loc[:])

            # ---- phase B: build zT from emb_i / emb_j ----
            for ei, stag in enumerate(stags):
                base = ei * N
                zbig = zc_pool.tile([128, 32, 128], bf16, tag="zbig",
                                    name="zbig")
                normalize(stag, 32, zbig[:])
                for g in range(8):
                    ps4 = pst_pool.tile([128, 4, 128], bf16, tag="ps4",
                                        name="ps4")
                    for dlt in range(4):
                        nc.tensor.transpose(ps4[:, dlt, :],
                                            zbig[:, 4 * g + dlt, :], ident_sb[:])
                    # chunks 4g..4g+3 transpose to zT columns
                    # [base+512g, base+512g+512), contiguous.
                    nc.vector.tensor_copy(
                        zT[:, base + 512 * g: base + 512 * (g + 1)]
                        .rearrange("q (n p) -> q n p", n=4),
                        ps4[:])

            # ---- phase C: G = Z_loc^T @ Z_loc (local Gram, 128x128) ----
            g_ps = psg_pool.tile([128, 128], f32)
            for m in range(M_BLK):
                nc.tensor.matmul(g_ps[:], lhsT=z_loc[:, m, :], rhs=z_loc[:, m, :],
                                 start=(m == 0), stop=(m == M_BLK - 1))
            g_sb = tmp_pool.tile([128, 128], f32, tag="gsb")
            nc.scalar.copy(g_sb[:], g_ps[:])
            nc.sync.dma_start(out=out_g[:], in_=g_sb[:])

            # ---- phase D: consume the class_pairs stream ----
            s_idx = 0
            v_idx = 0
            for gi, w in enumerate(GROUP_W):
                cbase = col_bases[gi]
                cpts = cp_tiles[gi]
                n_k = w // NCH
                # m-major emission: tile m's matmuls for ALL chunks run as
                # soon as tile m lands (and the stationary z_loc[m] weights
                # load once per tile instead of once per (chunk, tile)).
                # After the group's last tile, only n_k matmuls + dots remain.
                pss = [psv_pool.tile([128, NCH], f32, tag="psv", name="psv")
                       for _ in range(n_k)]
                for m in range(M_BLK):
                    for k in range(n_k):
                        nc.tensor.matmul(pss[k][:], lhsT=z_loc[:, m, :],
                                         rhs=cpts[m][:, k * NCH:(k + 1) * NCH],
                                         start=(m == 0), stop=(m == M_BLK - 1))
                for k in range(n_k):
                    col0 = cbase + k * NCH
                    xj = tmp_pool.tile([128, NCH], bf16, tag="xj", name="xj")
                    # acc_x[:, chunk] = sum_c ps[:, c] * zT[:, col0 + c]
                    xcol = col0 // NCH
                    nc.vector.scalar_tensor_tensor(
                        out=xj[:], in0=pss[k][:], scalar=1.0,
                        in1=zT[:, col0:col0 + NCH],
                        op0=OP.mult, op1=OP.mult,
                        accum_out=acc_x[:, xcol:xcol + 1])
                for m in range(M_BLK):
                    if SQ_ON_DVE[gi * M_BLK + m]:
                        sj = sqj_pool.tile([128, w], bf16, tag="sjv",
                                           name="sjv", padded_shape=[128, 2048])
                        nc.vector.scalar_tensor_tensor(
                            out=sj[:], in0=cpts[m][:], scalar=1.0,
                            in1=cpts[m][:], op0=OP.mult, op1=OP.mult,
                            accum_out=acc_v[:, v_idx:v_idx + 1])
                        v_idx += 1
                    else:
                        sj = sqj_pool.tile([128, w], bf16, tag="sja",
                                           name="sja", padded_shape=[128, 2048])
                        nc.scalar.activation(sj[:], cpts[m][:], AF.Square,
                                             accum_out=acc_s[:, s_idx:s_idx + 1])
                        s_idx += 1

            nc.sync.dma_start(out=out_s[:], in_=acc_s[:])
            nc.sync.dma_start(out=out_v[:], in_=acc_v[:])
            nc.sync.dma_start(out=out_x[:], in_=acc_x[:])

    nc.compile()
    return nc


def _get_module():
    if "nc" not in _cached:
        _cached["nc"] = _build_module()
    return _cached["nc"]


def kernel(emb_i, emb_j, class_pairs, _return_raw=False, _trace=False):
    import ml_dtypes

    emb_i = np.ascontiguousarray(emb_i, dtype=np.float32)
    emb_j = np.ascontiguousarray(emb_j, dtype=np.float32)
    class_pairs = np.ascontiguousarray(class_pairs, dtype=np.float32)
    ident = np.eye(128, dtype=ml_dtypes.bfloat16)

    def stage(a):
        # host-side shard layout: bf16 [partition, block, d] with block n
        # holding row n*128+p on partition p (see _build_module)
        n = a.shape[0] // 128
        return np.ascontiguousarray(
            a.astype(ml_dtypes.bfloat16).reshape(n, 128, D).transpose(1, 0, 2))

    emb_i_st = stage(emb_i)
    emb_j_st = stage(emb_j)

    nc = _get_module()
    in_maps = []
    for c in range(N_CORES):
        r0 = c * R_LOC
        if r0 < N:
            emb_loc = emb_i[r0:r0 + R_LOC]
        else:
            emb_loc = emb_j[r0 - N:r0 - N + R_LOC]
        in_maps.append({
            "emb_i": emb_i_st,
            "emb_j": emb_j_st,
            "emb_loc": stage(emb_loc),
            "cp_loc": np.ascontiguousarray(class_pairs[r0:r0 + R_LOC]),
            "ident": ident,
        })

    res = run_bass_kernel_spmd(nc, in_maps, list(range(N_CORES)), trace=_trace)

    G = np.zeros((128, 128), dtype=np.float64)
    sum_cp2 = 0.0
    cross = 0.0
    for c in range(N_CORES):
        r = res.results[c]
        G += r["out_g"].astype(np.float64)
        sum_cp2 += r["out_s"].astype(np.float64).sum()
        sum_cp2 += r["out_v"].astype(np.float64).sum()
        cross += r["out_x"].astype(np.float64).sum()
    sum_sim2 = float((G * G).sum())
    loss = (sum_sim2 - 2.0 * cross + sum_cp2) / float(TWO_N * TWO_N)
    out = np.asarray(loss, dtype=np.float32)
    if _return_raw:
        return out, res
    return out


# revision 8
# speedup vs baseline: 1.0020x; 1.0020x over previous
"""Trainium2 Bass kernel for nn_ContrastiveLossV2 (8-core SPMD).

Reference computation:
    z = l2norm(concat([emb_i, emb_j]))          # [8192, 128]
    sim = z @ z.T                               # [8192, 8192]
    loss = mean((sim - class_pairs)**2)

Instead of materializing sim and running an elementwise (sim-cp)^2 pass,
expand the square:

    sum((sim-cp)^2) = sum(sim^2) - 2*sum(sim*cp) + sum(cp^2)

  * sum(sim^2)  = ||Z^T Z||_F^2   (Gram trick; per-core G_c = Z_c^T Z_c over
                  its 1024 local rows, G = sum_c G_c summed on host in f64)
  * sum(sim*cp) = sum_{d,c} V[d,c] * zT[d,c]  with  V = Z_loc^T @ CP_loc —
                  computed by the TensorEngine directly from *row-major* cp
                  tiles (contraction over the local row dim). The dot against
                  zT is a fused scalar_tensor_tensor per PSUM chunk.
  * sum(cp^2)   = Square+accumulate passes, split between the scalar and
                  vector engines.

Row sharding: core c owns reps rows [c*1024, (c+1)*1024) and the matching
1024-row slice of class_pairs. Everything per-core-specific arrives as data,
so one SPMD program serves all 8 cores.

The kernel is bound by the per-NeuronCore HBM read limit (~360-390 GB/s):
32MB of class_pairs per core. Structure:
  * the 32MB class_pairs stream (SWDGE, f32->bf16 cast in flight) is the
    FIRST thing on the gpsimd queue; all small staging inputs (pre-staged
    bf16 embs, identity) load via the scalar-engine HWDGE queue in parallel.
  * column groups [2048,2048,2048,1024,1024]: full-width for bulk stream
    efficiency, narrower at the end so the post-stream compute tail is short.
  * each engine accumulates cp^2 partials into its OWN tile and the narrow
    groups alternate scalar/vector per tile. (A shared accumulator tile
    cross-engine-orders the accumulate ops and was measured to serialize the
    entire stream at ~297 GB/s.)
Host combines all partial tensors in f64.
"""

import numpy as np

import concourse.bacc as bacc
import concourse.mybir as mybir
import concourse.tile as tile
from concourse.bass_utils import run_bass_kernel_spmd

f32 = mybir.dt.float32
bf16 = mybir.dt.bfloat16
AF = mybir.ActivationFunctionType
OP = mybir.AluOpType

N_CORES = 8
N, D = 4096, 128
TWO_N = 2 * N                     # 8192
R_LOC = TWO_N // N_CORES          # 1024 rows per core
M_BLK = R_LOC // 128              # 8 local 128-row blocks
NCH = 512                         # matmul free-dim chunk (one PSUM bank)
GROUP_W = [2048, 2048, 2048, 1024, 1024]       # column group widths
N_GRP = len(GROUP_W)
N_CHUNK = TWO_N // NCH            # 16 psum chunks total
EPS = 1e-12
# how many of the 8 per-group cp^2 squares run on the vector engine in the
# full-width groups (the rest run on the scalar engine)
CP2_ON_DVE = 2

# per-(group, m) engine assignment for the cp^2 squares: full-width groups
# put CP2_ON_DVE of 8 on the vector engine, narrow groups alternate per tile
# so neither engine serializes the stream tail.
SQ_ON_DVE = []
for _w in GROUP_W:
    for _m in range(M_BLK):
        SQ_ON_DVE.append(_m < CP2_ON_DVE)
N_SQ_V = sum(SQ_ON_DVE)
N_SQ_S = len(SQ_ON_DVE) - N_SQ_V

_cached = {}


def _build_module():
    nc = bacc.Bacc("TRN2", target_bir_lowering=False, debug=False,
                   num_devices=N_CORES)

    # The (small, replicated) embedding inputs are uploaded pre-staged by the
    # host: bf16, already arranged as [partition, block, d] with block n
    # holding row n*128+p on partition p — so the device DMA is a fully
    # contiguous per-partition read instead of a 512B/row gather.
    emb_i = nc.dram_tensor("emb_i", [128, N // 128, D], bf16,
                           kind="ExternalInput")
    emb_j = nc.dram_tensor("emb_j", [128, N // 128, D], bf16,
                           kind="ExternalInput")
    emb_loc = nc.dram_tensor("emb_loc", [128, M_BLK, D], bf16,
                             kind="ExternalInput")
    cp_loc = nc.dram_tensor("cp_loc", [R_LOC, TWO_N], f32, kind="ExternalInput")
    ident = nc.dram_tensor("ident", [128, 128], bf16, kind="ExternalInput")

    out_g = nc.dram_tensor("out_g", [128, 128], f32, kind="ExternalOutput")
    out_s = nc.dram_tensor("out_s", [128, N_SQ_S], f32, kind="ExternalOutput")
    out_v = nc.dram_tensor("out_v", [128, N_SQ_V], f32, kind="ExternalOutput")
    out_x = nc.dram_tensor("out_x", [128, N_CHUNK], f32, kind="ExternalOutput")

    with tile.TileContext(nc) as tc:
        with (
            tc.tile_pool(name="const", bufs=1) as const_pool,
            tc.tile_pool(name="persist", bufs=1) as persist,
            tc.tile_pool(name="stag", bufs=2) as stag_pool,
            tc.tile_pool(name="sq", bufs=1) as sq_pool,
            tc.tile_pool(name="norm", bufs=4) as norm_pool,
            tc.tile_pool(name="zc", bufs=2) as zc_pool,
            tc.tile_pool(name="cpt", bufs=22) as cp_pool,
            tc.tile_pool(name="tmp", bufs=4) as tmp_pool,
            tc.tile_pool(name="sqj", bufs=2) as sqj_pool,
            tc.tile_pool(name="psv", bufs=5, space="PSUM") as psv_pool,
            tc.tile_pool(name="pst", bufs=2, space="PSUM") as pst_pool,
            tc.tile_pool(name="psg", bufs=1, space="PSUM") as psg_pool,
        ):
            # zT[d, r] = normalized reps row r, transposed. bf16.
            zT = persist.tile([128, TWO_N], bf16)
            # local row-major z tiles (natural 128-row blocks), bf16.
            z_loc = persist.tile([128, M_BLK, 128], bf16)
            # per-engine partial-sum accumulators (separate tiles: a shared
            # tile would cross-engine-order the accumulate ops)
            acc_s = persist.tile([128, N_SQ_S], f32)
            acc_v = persist.tile([128, N_SQ_V], f32)
            acc_x = persist.tile([128, N_CHUNK], f32)

            # ---- class_pairs stream: FIRST ops on the SWDGE queue ----
            # (cast f32->bf16 in flight; descriptor generation for tile 0
            # begins the moment the gpsimd engine is up)
            cp_tiles = [[None] * M_BLK for _ in GROUP_W]
            col_bases = []
            cb = 0
            for w in GROUP_W:
                col_bases.append(cb)
                cb += w
            for gi, w in enumerate(GROUP_W):
                cbase = col_bases[gi]
                for m in range(M_BLK):
                    cpt = cp_pool.tile([128, w], bf16, tag="cpt",
                                       name="cpt", padded_shape=[128, 2048])
                    nc.gpsimd.dma_start(
                        out=cpt[:],
                        in_=cp_loc[m * 128:(m + 1) * 128, cbase:cbase + w])
                    cp_tiles[gi][m] = cpt

            # ---- staging inputs via the scalar-engine HWDGE queue ----
            stag_l = stag_pool.tile([128, M_BLK, 128], bf16, tag="stag_loc",
                                    name="stag_loc")
            nc.scalar.dma_start(out=stag_l[:], in_=emb_loc[:])
            stags = []
            for emb in (emb_i, emb_j):
                stag = stag_pool.tile([128, 32, 128], bf16, tag="stag",
                                      name="stag")
                nc.scalar.dma_start(out=stag[:], in_=emb[:])
                stags.append(stag)
            ident_sb = const_pool.tile([128, 128], bf16)
            nc.scalar.dma_start(out=ident_sb[:], in_=ident[:])

            def normalize(stag, n_chunks, z_out):
                """stag: [128, n_chunks, 128] bf16 staging; chunk n is the
                natural 128-row block n (row n*128+p on partition p). Writes
                the row-normalized bf16 result into z_out [128, n_chunks, 128]
                with a single broadcast multiply (f32 internal math)."""
                sq = sq_pool.tile([128, n_chunks, 128], f32, tag="sq",
                                  name="sq")
                nc.vector.tensor_tensor(sq[:], stag[:], stag[:], op=OP.mult)
                nsq = norm_pool.tile([128, n_chunks], f32, tag="nsq",
                                     name="nsq")
                nc.vector.tensor_reduce(nsq[:], sq[:], axis=mybir.AxisListType.X,
                                        op=OP.add)
                nrm = norm_pool.tile([128, n_chunks], f32, tag="nrm",
                                     name="nrm")
                nc.scalar.activation(nrm[:], nsq[:], AF.Sqrt)
                nc.vector.tensor_scalar_max(nrm[:], nrm[:], EPS)
                rec = norm_pool.tile([128, n_chunks], f32, tag="rec",
                                     name="rec")
                nc.vector.reciprocal(rec[:], nrm[:])
                rec_b = rec[:].rearrange("q n -> q n ()") \
                    .broadcast_to([128, n_chunks, 128])
                nc.vector.tensor_tensor(z_out, stag[:], rec_b, op=OP.mult)

            # ---- phase A: local row blocks (natural order) ----
            normalize(stag_l, M_BLK, z_loc[:])

            # ---- phase B: build zT from emb_i / emb_j ----
            for ei, stag in enumerate(stags):
                base = ei * N
                zbig = zc_pool.tile([128, 32, 128], bf16, tag="zbig",
                                    name="zbig")
                normalize(stag, 32, zbig[:])
                for g in range(8):
                    ps4 = pst_pool.tile([128, 4, 128], bf16, tag="ps4",
                                        name="ps4")
                    for dlt in range(4):
                        nc.tensor.transpose(ps4[:, dlt, :],
                                            zbig[:, 4 * g + dlt, :], ident_sb[:])
                    # chunks 4g..4g+3 transpose to zT columns
                    # [base+512g, base+512g+512), contiguous.
                    nc.vector.tensor_copy(
                        zT[:, base + 512 * g: base + 512 * (g + 1)]
                        .rearrange("q (n p) -> q n p", n=4),
                        ps4[:])

            # ---- phase C: G = Z_loc^T @ Z_loc (local Gram, 128x128) ----
            g_ps = psg_pool.tile([128, 128], f32)
            for m in range(M_BLK):
                nc.tensor.matmul(g_ps[:], lhsT=z_loc[:, m, :], rhs=z_loc[:, m, :],
                                 start=(m == 0), stop=(m == M_BLK - 1))
            g_sb = tmp_pool.tile([128, 128], f32, tag="gsb")
            nc.scalar.copy(g_sb[:], g_ps[:])
            nc.sync.dma_start(out=out_g[:], in_=g_sb[:])

            # ---- phase D: consume the class_pairs stream ----
            s_idx = 0
            v_idx = 0
            for gi, w in enumerate(GROUP_W):
                cbase = col_bases[gi]
                cpts = cp_tiles[gi]
                n_k = w // NCH
                # m-major emission: tile m's matmuls for ALL chunks run as
                # soon as tile m lands (and the stationary z_loc[m] weights
                # load once per tile instead of once per (chunk, tile)).
                # After the group's last tile, only n_k matmuls + dots remain.
                pss = [psv_pool.tile([128, NCH], f32, tag="psv", name="psv")
                       for _ in range(n_k)]
                for m in range(M_BLK):
                    for k in range(n_k):
                        nc.tensor.matmul(pss[k][:], lhsT=z_loc[:, m, :],
                                         rhs=cpts[m][:, k * NCH:(k + 1) * NCH],
                                         start=(m == 0), stop=(m == M_BLK - 1))
                for k in range(n_k):
                    col0 = cbase + k * NCH
                    xj = tmp_pool.tile([128, NCH], bf16, tag="xj", name="xj")
                    # acc_x[:, chunk] = sum_c ps[:, c] * zT[:, col0 + c]
                    xcol = col0 // NCH
                    nc.vector.scalar_tensor_tensor(
                        out=xj[:], in0=pss[k][:], scalar=1.0,
                        in1=zT[:, col0:col0 + NCH],
                        op0=OP.mult, op1=OP.mult,
                        accum_out=acc_x[:, xcol:xcol + 1])
                for m in range(M_BLK):
                    if SQ_ON_DVE[gi * M_BLK + m]:
                        sj = sqj_pool.tile([128, w], bf16, tag="sjv",
                                           name="sjv", padded_shape=[128, 2048])
                        nc.vector.scalar_tensor_tensor(
                            out=sj[:], in0=cpts[m][:], scalar=1.0,
                            in1=cpts[m][:], op0=OP.mult, op1=OP.mult,
                            accum_out=acc_v[:, v_idx:v_idx + 1])
                        v_idx += 1
                    else:
                        sj = sqj_pool.tile([128, w], bf16, tag="sja",
                                           name="sja", padded_shape=[128, 2048])
                        nc.scalar.activation(sj[:], cpts[m][:], AF.Square,
                                             accum_out=acc_s[:, s_idx:s_idx + 1])
                        s_idx += 1

            nc.sync.dma_start(out=out_s[:], in_=acc_s[:])
            nc.sync.dma_start(out=out_v[:], in_=acc_v[:])
            nc.sync.dma_start(out=out_x[:], in_=acc_x[:])

    nc.compile()
    return nc


def _get_module():
    if "nc" not in _cached:
        _cached["nc"] = _build_module()
    return _cached["nc"]


def kernel(emb_i, emb_j, class_pairs, _return_raw=False, _trace=False):
    import ml_dtypes

    emb_i = np.ascontiguousarray(emb_i, dtype=np.float32)
    emb_j = np.ascontiguousarray(emb_j, dtype=np.float32)
    class_pairs = np.ascontiguousarray(class_pairs, dtype=np.float32)
    ident = np.eye(128, dtype=ml_dtypes.bfloat16)

    def stage(a):
        # host-side shard layout: bf16 [partition, block, d] with block n
        # holding row n*128+p on partition p (see _build_module)
        n = a.shape[0] // 128
        return np.ascontiguousarray(
            a.astype(ml_dtypes.bfloat16).reshape(n, 128, D).transpose(1, 0, 2))

    emb_i_st = stage(emb_i)
    emb_j_st = stage(emb_j)

    nc = _get_module()
    in_maps = []
    for c in range(N_CORES):
        r0 = c * R_LOC
        if r0 < N:
            emb_loc = emb_i[r0:r0 + R_LOC]
        else:
            emb_loc = emb_j[r0 - N:r0 - N + R_LOC]
        in_maps.append({
            "emb_i": emb_i_st,
            "emb_j": emb_j_st,
            "emb_loc": stage(emb_loc),
            "cp_loc": np.ascontiguousarray(class_pairs[r0:r0 + R_LOC]),
            "ident": ident,
        })

    res = run_bass_kernel_spmd(nc, in_maps, list(range(N_CORES)), trace=_trace)

    G = np.zeros((128, 128), dtype=np.float64)
    sum_cp2 = 0.0
    cross = 0.0
    for c in range(N_CORES):
        r = res.results[c]
        G += r["out_g"].astype(np.float64)
        sum_cp2 += r["out_s"].astype(np.float64).sum()
        sum_cp2 += r["out_v"].astype(np.float64).sum()
        cross += r["out_x"].astype(np.float64).sum()
    sum_sim2 = float((G * G).sum())
    loss = (sum_sim2 - 2.0 * cross + sum_cp2) / float(TWO_N * TWO_N)
    out = np.asarray(loss, dtype=np.float32)
    if _return_raw:
        return out, res
    return out


# revision 14
# speedup vs baseline: 1.1614x; 1.1591x over previous
"""Trainium2 Bass kernel for nn_ContrastiveLossV2 (8-core SPMD).

Reference computation:
    z = l2norm(concat([emb_i, emb_j]))          # [8192, 128]
    sim = z @ z.T                               # [8192, 8192]
    loss = mean((sim - class_pairs)**2)

Instead of materializing sim and running an elementwise (sim-cp)^2 pass,
expand the square:

    sum((sim-cp)^2) = sum(sim^2) - 2*sum(sim*cp) + sum(cp^2)

  * sum(sim^2)  = ||Z^T Z||_F^2   (Gram trick; per-core G_c = Z_c^T Z_c over
                  its 1024 local rows, G = sum_c G_c summed on host in f64)
  * sum(sim*cp) = sum_{d,c} V[d,c] * zT[d,c]  with  V = Z_loc^T @ CP_loc —
                  computed by the TensorEngine directly from *row-major* cp
                  tiles (contraction over the local row dim). The dot against
                  zT is a fused scalar_tensor_tensor per PSUM chunk.
  * sum(cp^2)   = Square+accumulate passes, split between the scalar and
                  vector engines.

Row sharding: core c owns reps rows [c*1024, (c+1)*1024) and the matching
1024-row slice of class_pairs. Everything per-core-specific arrives as data,
so one SPMD program serves all 8 cores.

The kernel is bound by the per-NeuronCore HBM read limit (~360-390 GB/s):
32MB of class_pairs per core. Structure:
  * the 32MB class_pairs stream (SWDGE, f32->bf16 cast in flight) is the
    FIRST thing on the gpsimd queue; all small staging inputs (pre-staged
    bf16 embs, identity) load via the scalar-engine HWDGE queue in parallel.
  * column groups [2048,2048,2048,1024,1024]: full-width for bulk stream
    efficiency, narrower at the end so the post-stream compute tail is short.
  * each engine accumulates cp^2 partials into its OWN tile and the narrow
    groups alternate scalar/vector per tile. (A shared accumulator tile
    cross-engine-orders the accumulate ops and was measured to serialize the
    entire stream at ~297 GB/s.)
Host combines all partial tensors in f64.
"""

import numpy as np

import concourse.bacc as bacc
import concourse.mybir as mybir
import concourse.tile as tile
from concourse.bass_utils import run_bass_kernel_spmd

f32 = mybir.dt.float32
bf16 = mybir.dt.bfloat16
AF = mybir.ActivationFunctionType
OP = mybir.AluOpType

N_CORES = 8
N, D = 4096, 128
TWO_N = 2 * N                     # 8192
R_LOC = TWO_N // N_CORES          # 1024 rows per core
M_BLK = R_LOC // 128              # 8 local 128-row blocks
NCH = 512                         # matmul free-dim chunk (one PSUM bank)
GROUP_W = [2048, 2048, 2048, 1024, 1024]       # column group widths
N_GRP = len(GROUP_W)
N_CHUNK = TWO_N // NCH            # 16 psum chunks total
EPS = 1e-12
# how many of the 8 per-group cp^2 squares run on the vector engine in the
# full-width groups (the rest run on the scalar engine)
CP2_ON_DVE = 2

# per-(group, m) engine assignment for the cp^2 squares: full-width groups
# put CP2_ON_DVE of 8 on the vector engine, narrow groups alternate per tile
# so neither engine serializes the stream tail.
SQ_ON_DVE = []
for _w in GROUP_W:
    for _m in range(M_BLK):
        SQ_ON_DVE.append(_m < CP2_ON_DVE)
N_SQ_V = sum(SQ_ON_DVE)
N_SQ_S = len(SQ_ON_DVE) - N_SQ_V

_cached = {}


def _build_module():
    nc = bacc.Bacc("TRN2", target_bir_lowering=False, debug=False,
                   num_devices=N_CORES)

    # The (small, replicated) embedding inputs are uploaded pre-staged by the
    # host: bf16, already arranged as [partition, block, d] with block n
    # holding row n*128+p on partition p — so the device DMA is a fully
    # contiguous per-partition read instead of a 512B/row gather.
    emb_i = nc.dram_tensor("emb_i", [128, N // 128, D], bf16,
                           kind="ExternalInput")
    emb_j = nc.dram_tensor("emb_j", [128, N // 128, D], bf16,
                           kind="ExternalInput")
    emb_loc = nc.dram_tensor("emb_loc", [128, M_BLK, D], bf16,
                             kind="ExternalInput")
    cp_loc = nc.dram_tensor("cp_loc", [R_LOC, TWO_N], f32, kind="ExternalInput")
    ident = nc.dram_tensor("ident", [128, 128], bf16, kind="ExternalInput")

    # two packed outputs, one per accumulating engine: scalar-engine partials
    # (cp^2 squares + Gram G), vector-engine partials (cp^2 squares + cross)
    out_sc = nc.dram_tensor("out_sc", [128, N_SQ_S + 128], f32,
                            kind="ExternalOutput")
    out_vx = nc.dram_tensor("out_vx", [128, N_SQ_V + N_CHUNK], f32,
                            kind="ExternalOutput")

    with tile.TileContext(nc) as tc:
        with (
            tc.tile_pool(name="const", bufs=1) as const_pool,
            tc.tile_pool(name="persist", bufs=1) as persist,
            tc.tile_pool(name="stag", bufs=2) as stag_pool,
            tc.tile_pool(name="sq", bufs=1) as sq_pool,
            tc.tile_pool(name="norm", bufs=4) as norm_pool,
            tc.tile_pool(name="zc", bufs=2) as zc_pool,
            tc.tile_pool(name="cpt", bufs=22) as cp_pool,
            tc.tile_pool(name="tmp", bufs=4) as tmp_pool,
            tc.tile_pool(name="sqj", bufs=2) as sqj_pool,
            tc.tile_pool(name="psv", bufs=5, space="PSUM") as psv_pool,
            tc.tile_pool(name="pst", bufs=2, space="PSUM") as pst_pool,
            tc.tile_pool(name="psg", bufs=1, space="PSUM") as psg_pool,
        ):
            # zT[d, r] = normalized reps row r, transposed. bf16.
            zT = persist.tile([128, TWO_N], bf16)
            # local row-major z tiles (natural 128-row blocks), bf16.
            z_loc = persist.tile([128, M_BLK, 128], bf16)
            # per-engine partial-sum accumulators (separate tiles per engine:
            # a cross-engine shared tile orders the accumulate ops and was
            # measured to serialize the entire stream)
            acc_sc = persist.tile([128, N_SQ_S + 128], f32)
            acc_vx = persist.tile([128, N_SQ_V + N_CHUNK], f32)

            # ---- class_pairs stream: FIRST ops on the SWDGE queue ----
            # (cast f32->bf16 in flight; descriptor generation for tile 0
            # begins the moment the gpsimd engine is up)
            cp_tiles = [[None] * M_BLK for _ in GROUP_W]
            col_bases = []
            cb = 0
            for w in GROUP_W:
                col_bases.append(cb)
                cb += w
            for gi, w in enumerate(GROUP_W):
                cbase = col_bases[gi]
                for m in range(M_BLK):
                    cpt = cp_pool.tile([128, w], bf16, tag="cpt",
                                       name="cpt", padded_shape=[128, 2048])
                    nc.gpsimd.dma_start(
                        out=cpt[:],
                        in_=cp_loc[m * 128:(m + 1) * 128, cbase:cbase + w])
                    cp_tiles[gi][m] = cpt

            # ---- staging inputs via the scalar-engine HWDGE queue ----
            stag_l = stag_pool.tile([128, M_BLK, 128], bf16, tag="stag_loc",
                                    name="stag_loc")
            nc.scalar.dma_start(out=stag_l[:], in_=emb_loc[:])
            stags = []
            for emb in (emb_i, emb_j):
                stag = stag_pool.tile([128, 32, 128], bf16, tag="stag",
                                      name="stag")
                nc.scalar.dma_start(out=stag[:], in_=emb[:])
                stags.append(stag)
            ident_sb = const_pool.tile([128, 128], bf16)
            nc.scalar.dma_start(out=ident_sb[:], in_=ident[:])

            def normalize(stag, n_chunks, z_out):
                """stag: [128, n_chunks, 128] bf16 staging; chunk n is the
                natural 128-row block n (row n*128+p on partition p). Writes
                the row-normalized bf16 result into z_out [128, n_chunks, 128]
                with a single broadcast multiply (f32 internal math)."""
                sq = sq_pool.tile([128, n_chunks, 128], f32, tag="sq",
                                  name="sq")
                nc.vector.tensor_tensor(sq[:], stag[:], stag[:], op=OP.mult)
                nsq = norm_pool.tile([128, n_chunks], f32, tag="nsq",
                                     name="nsq")
                nc.vector.tensor_reduce(nsq[:], sq[:], axis=mybir.AxisListType.X,
                                        op=OP.add)
                nrm = norm_pool.tile([128, n_chunks], f32, tag="nrm",
                                     name="nrm")
                nc.scalar.activation(nrm[:], nsq[:], AF.Sqrt)
                nc.vector.tensor_scalar_max(nrm[:], nrm[:], EPS)
                rec = norm_pool.tile([128, n_chunks], f32, tag="rec",
                                     name="rec")
                nc.vector.reciprocal(rec[:], nrm[:])
                rec_b = rec[:].rearrange("q n -> q n ()") \
                    .broadcast_to([128, n_chunks, 128])
                nc.vector.tensor_tensor(z_out, stag[:], rec_b, op=OP.mult)

            # ---- phase A: local row blocks (natural order) ----
            normalize(stag_l, M_BLK, z_loc[:])

            # ---- phase B: build zT from emb_i / emb_j ----
            for ei, stag in enumerate(stags):
                base = ei * N
                zbig = zc_pool.tile([128, 32, 128], bf16, tag="zbig",
                                    name="zbig")
                normalize(stag, 32, zbig[:])
                for g in range(8):
                    ps4 = pst_pool.tile([128, 4, 128], bf16, tag="ps4",
                                        name="ps4")
                    for dlt in range(4):
                        nc.tensor.transpose(ps4[:, dlt, :],
                                            zbig[:, 4 * g + dlt, :], ident_sb[:])
                    # chunks 4g..4g+3 transpose to zT columns
                    # [base+512g, base+512g+512), contiguous.
                    nc.vector.tensor_copy(
                        zT[:, base + 512 * g: base + 512 * (g + 1)]
                        .rearrange("q (n p) -> q n p", n=4),
                        ps4[:])

            # ---- phase C: G = Z_loc^T @ Z_loc (local Gram, 128x128) ----
            g_ps = psg_pool.tile([128, 128], f32)
            for m in range(M_BLK):
                nc.tensor.matmul(g_ps[:], lhsT=z_loc[:, m, :], rhs=z_loc[:, m, :],
                                 start=(m == 0), stop=(m == M_BLK - 1))
            nc.scalar.copy(acc_sc[:, N_SQ_S:N_SQ_S + 128], g_ps[:])

            # ---- phase D: consume the class_pairs stream ----
            s_idx = 0
            v_idx = 0
            for gi, w in enumerate(GROUP_W):
                cbase = col_bases[gi]
                cpts = cp_tiles[gi]
                n_k = w // NCH
                last_grp = (gi == N_GRP - 1)
                if not last_grp:
                    # k-major: each chunk's 8-matmul PSUM chain runs back to
                    # back (PE stays warm; PSUM bank fixed per chain — bank
                    # cycling per matmul was measured to re-throttle the PE)
                    for k in range(n_k):
                        ps = psv_pool.tile([128, NCH], f32, tag="psv",
                                           name="psv")
                        for m in range(M_BLK):
                            nc.tensor.matmul(
                                ps[:], lhsT=z_loc[:, m, :],
                                rhs=cpts[m][:, k * NCH:(k + 1) * NCH],
                                start=(m == 0), stop=(m == M_BLK - 1))
                        col0 = cbase + k * NCH
                        xj = tmp_pool.tile([128, NCH], bf16, tag="xj",
                                           name="xj")
                        xcol = col0 // NCH
                        nc.vector.scalar_tensor_tensor(
                            out=xj[:], in0=ps[:], scalar=1.0,
                            in1=zT[:, col0:col0 + NCH],
                            op0=OP.mult, op1=OP.mult,
                            accum_out=acc_vx[:, N_SQ_V + xcol:
                                             N_SQ_V + xcol + 1])
                else:
                    # last group m-major: tile m's matmuls run on arrival, so
                    # after the final tile lands only n_k matmuls + dots
                    # remain before the output DMA
                    pss = [psv_pool.tile([128, NCH], f32, tag="psv",
                                         name="psv") for _ in range(n_k)]
                    for m in range(M_BLK):
                        for k in range(n_k):
                            nc.tensor.matmul(
                                pss[k][:], lhsT=z_loc[:, m, :],
                                rhs=cpts[m][:, k * NCH:(k + 1) * NCH],
                                start=(m == 0), stop=(m == M_BLK - 1))
                    for k in range(n_k):
                        col0 = cbase + k * NCH
                        xj = tmp_pool.tile([128, NCH], bf16, tag="xj",
                                           name="xj")
                        xcol = col0 // NCH
                        nc.vector.scalar_tensor_tensor(
                            out=xj[:], in0=pss[k][:], scalar=1.0,
                            in1=zT[:, col0:col0 + NCH],
                            op0=OP.mult, op1=OP.mult,
                            accum_out=acc_vx[:, N_SQ_V + xcol:
                                             N_SQ_V + xcol + 1])
                for m in range(M_BLK):
                    if SQ_ON_DVE[gi * M_BLK + m]:
                        sj = sqj_pool.tile([128, w], bf16, tag="sjv",
                                           name="sjv", padded_shape=[128, 2048])
                        nc.vector.scalar_tensor_tensor(
                            out=sj[:], in0=cpts[m][:], scalar=1.0,
                            in1=cpts[m][:], op0=OP.mult, op1=OP.mult,
                            accum_out=acc_vx[:, v_idx:v_idx + 1])
                        v_idx += 1
                    else:
                        sj = sqj_pool.tile([128, w], bf16, tag="sja",
                                           name="sja", padded_shape=[128, 2048])
                        nc.scalar.activation(sj[:], cpts[m][:], AF.Square,
                                             accum_out=acc_sc[:, s_idx:s_idx + 1])
                        s_idx += 1

            nc.sync.dma_start(out=out_sc[:], in_=acc_sc[:])
            nc.sync.dma_start(out=out_vx[:], in_=acc_vx[:])

    nc.compile()
    return nc


def _get_module():
    if "nc" not in _cached:
        _cached["nc"] = _build_module()
    return _cached["nc"]


def kernel(emb_i, emb_j, class_pairs, _return_raw=False, _trace=False):
    import ml_dtypes

    emb_i = np.ascontiguousarray(emb_i, dtype=np.float32)
    emb_j = np.ascontiguousarray(emb_j, dtype=np.float32)
    class_pairs = np.ascontiguousarray(class_pairs, dtype=np.float32)
    ident = np.eye(128, dtype=ml_dtypes.bfloat16)

    def stage(a):
        # host-side shard layout: bf16 [partition, block, d] with block n
        # holding row n*128+p on partition p (see _build_module)
        n = a.shape[0] // 128
        return np.ascontiguousarray(
            a.astype(ml_dtypes.bfloat16).reshape(n, 128, D).transpose(1, 0, 2))

    emb_i_st = stage(emb_i)
    emb_j_st = stage(emb_j)

    nc = _get_module()
    in_maps = []
    for c in range(N_CORES):
        r0 = c * R_LOC
        if r0 < N:
            emb_loc = emb_i[r0:r0 + R_LOC]
        else:
            emb_loc = emb_j[r0 - N:r0 - N + R_LOC]
        in_maps.append({
            "emb_i": emb_i_st,
            "emb_j": emb_j_st,
            "emb_loc": stage(emb_loc),
            "cp_loc": np.ascontiguousarray(class_pairs[r0:r0 + R_LOC]),
            "ident": ident,
        })

    res = run_bass_kernel_spmd(nc, in_maps, list(range(N_CORES)), trace=_trace)

    G = np.zeros((128, 128), dtype=np.float64)
    sum_cp2 = 0.0
    cross = 0.0
    for c in range(N_CORES):
        r = res.results[c]
        sc = r["out_sc"].astype(np.float64)
        vx = r["out_vx"].astype(np.float64)
        G += sc[:, N_SQ_S:N_SQ_S + 128]
        sum_cp2 += sc[:, :N_SQ_S].sum()
        sum_cp2 += vx[:, :N_SQ_V].sum()
        cross += vx[:, N_SQ_V:N_SQ_V + N_CHUNK].sum()
    sum_sim2 = float((G * G).sum())
    loss = (sum_sim2 - 2.0 * cross + sum_cp2) / float(TWO_N * TWO_N)
    out = np.asarray(loss, dtype=np.float32)
    if _return_raw:
        return out, res
    return out
